# revision 1
# baseline (speedup 1.0000x reference)
"""Trainium2 Bass kernel for the attention-gate block.

Math (per sample n, after folding BN into the convs):
  X     = x[n, :, ::2, ::2].reshape(C, 4)                 # C=512, L=4
  act_k = relu(Wk' @ X + bk')            k=0,1,2          # D=64 each
  S     = act0^T act1  (4x4);  P = softmax_rows(S)
  Z     = P @ act2^T  (4x64)
  Y     = W4' @ Z^T + b4'                                  # (512, 4)
  out[n,c,h,w] = x[n,c,h,w] + Y[c,h]                       # broadcast over w

Device mapping (per core, 256 samples, blocks of 64):
  - channel packing c = 4p + j (p = partition, j = 0..3): each
    (partition, sample) moves one 256B-contiguous run, so a block is ONE
    big DMA each way (loads on the sync HWDGE queue, stores on scalar's).
    Weights are permuted host-side to match, so compute is unchanged.
  - GEMM1 computes q and k over 4 contraction groups; v is computed
    directly transposed ([samples*4 parts, d]) by swapping matmul
    operands, with its bias folded in via a K=1 ones-row matmul.
  - attention scores for 32 samples at a time come from one [64]x[128,128]
    gram matmul whose block-diagonal 4x4 blocks are the real scores;
    masked exp (ACT + 0/1 block-diag mask on DVE); softmax denominators
    via a ones-column matmul; normalization deferred past the P@V matmul.
  - GEMM2 (BN folded) does the w-broadcast in its rhs AP (step-0 re-read
    of each z column) so the residual add runs on plain stride-1 APs.
"""

import os
import sys

for _p in ("/opt/trn_rl_repo",):
    if _p not in sys.path:
        sys.path.insert(0, _p)

import numpy as np

import concourse.mybir as mybir
from concourse import bacc, tile

EPS = 1e-5
N_TOTAL, C, D, HH, WW = 2048, 512, 64, 4, 4
NCORES = 8
NSH = N_TOTAL // NCORES  # 256 samples per core
BLK = int(os.environ.get("KBLK", "128"))  # samples per block
SUB = 32                 # samples per attention subchunk (4*SUB = 128 cols)
SHIFT = -34.0            # constant exp shift; cancels in the normalization
F32 = mybir.dt.float32

_PROG_CACHE = {}


def build_program(nsh=NSH, blk=BLK, reps=1):
    key = (nsh, blk, reps)
    if key in _PROG_CACHE:
        return _PROG_CACHE[key]

    nc = bacc.Bacc("TRN2", target_bir_lowering=False, debug=False)
    AF = mybir.ActivationFunctionType

    x_in = nc.dram_tensor("x", (nsh, C, HH, WW), F32, kind="ExternalInput")
    wqk = nc.dram_tensor("wqk", (C, 128), F32, kind="ExternalInput")
    bqk = nc.dram_tensor("bqk", (128, 1), F32, kind="ExternalInput")
    w2a = nc.dram_tensor("w2a", (C, D), F32, kind="ExternalInput")
    b2a = nc.dram_tensor("b2a", (1, D), F32, kind="ExternalInput")
    w4t = nc.dram_tensor("w4t", (D, C), F32, kind="ExternalInput")
    b4v = nc.dram_tensor("b4v", (1, C), F32, kind="ExternalInput")
    msk = nc.dram_tensor("msk", (128, 128), F32, kind="ExternalInput")
    out = nc.dram_tensor("out", (nsh, C, HH, WW), F32, kind="ExternalOutput")

    nblk = nsh // blk
    nsub = blk // SUB
    NF = 4 * blk  # free width of a full block of (n, l) columns

    with tile.TileContext(nc) as tc:
        with (
            tc.tile_pool(name="const", bufs=1) as cpool,
            tc.tile_pool(name="xp", bufs=(3 if blk >= 128 else 4)) as xpool,
            tc.tile_pool(name="work", bufs=4) as wpool,
            tc.tile_pool(name="att", bufs=6) as apool,
            tc.tile_pool(name="ps", bufs=6, space="PSUM") as pspool,
            tc.tile_pool(name="psy", bufs=2, space="PSUM") as pypool,
        ):
            wq_sb = cpool.tile([128, 4, D], F32)
            nc.sync.dma_start(
                wq_sb[:], wqk[:, 0:D].rearrange("(k p) d -> p k d", p=128))
            wk_sb = cpool.tile([128, 4, D], F32)
            nc.sync.dma_start(
                wk_sb[:], wqk[:, D:2 * D].rearrange("(k p) d -> p k d", p=128))
            bq_sb = cpool.tile([D, 1], F32)
            nc.sync.dma_start(bq_sb[:], bqk[0:D])
            bk_sb = cpool.tile([D, 1], F32)
            nc.sync.dma_start(bk_sb[:], bqk[D:2 * D])
            w2a_sb = cpool.tile([128, 4, D], F32)
            nc.sync.dma_start(w2a_sb[:], w2a[:].rearrange("(k p) d -> p k d", p=128))
            b2a_sb = cpool.tile([1, D], F32)
            nc.sync.dma_start(b2a_sb[:], b2a[:])
            w4t_sb = cpool.tile([D, 4, 128], F32)
            nc.sync.dma_start(w4t_sb[:], w4t[:].rearrange("d (k p) -> d k p", p=128))
            b4c_sb = cpool.tile([128, 4], F32)
            nc.sync.dma_start(
                b4c_sb[:], b4v[:].rearrange("x (j p) -> p (x j)", j=4))
            msk_sb = cpool.tile([128, 128], F32)
            nc.sync.dma_start(msk_sb[:], msk[:])
            ones_sb = cpool.tile([1, max(NF, 512)], F32)
            nc.vector.memset(ones_sb[:], 1.0)
            ones_col = cpool.tile([128, 1], F32)
            nc.vector.memset(ones_col[:], 1.0)
            shift_sb = cpool.tile([128, 1], F32)
            nc.vector.memset(shift_sb[:], SHIFT)

            # channel packing c = 4p + j: one DMA per block each way
            xv = x_in[:].rearrange("(b n) (p j) h w -> b p n (j h w)", j=4, n=blk)
            ov = out[:].rearrange("(b n) (p j) h w -> b p n (j h w)", j=4, n=blk)

            for b in [b for _ in range(reps) for b in range(nblk)]:
                x_t = xpool.tile([128, blk, 64], F32, tag="x")
                nc.sync.dma_start(x_t[:], xv[b])
                xtv = x_t[:].rearrange("p n (j h w) -> p n j h w", j=4, h=4)

                # gather the ::2,::2 columns -> [128, j, n, l] with l=(h',w')
                xr = wpool.tile([128, 4, blk, 4], F32, tag="xr")
                nc.vector.tensor_copy(
                    xr[:].rearrange("p j n (a c) -> p j n a c", a=2),
                    xtv[:, :, :, 0:4:2, 0:4:2].transpose([0, 2, 1, 3, 4]),
                )
                xrf = xr[:].rearrange("p j n l -> p j (n l)")

                # GEMM1 q and k: [c=512 contraction] -> psum [64, NF] each
                ps_q = pspool.tile([D, NF], F32, tag="ps")
                ps_k = pspool.tile([D, NF], F32, tag="ps")
                for j in range(4):
                    nc.tensor.matmul(
                        ps_q[:], lhsT=wq_sb[:, j], rhs=xrf[:, j],
                        start=(j == 0), stop=(j == 3),
                    )
                for j in range(4):
                    nc.tensor.matmul(
                        ps_k[:], lhsT=wk_sb[:, j], rhs=xrf[:, j],
                        start=(j == 0), stop=(j == 3),
                    )
                a_q = wpool.tile([D, NF], F32, tag="aq")
                nc.scalar.activation(a_q[:], ps_q[:], AF.Relu, bias=bq_sb[:])
                a_k = wpool.tile([D, NF], F32, tag="ak")
                nc.scalar.activation(a_k[:], ps_k[:], AF.Relu, bias=bk_sb[:])

                # phase 1: independent PE work for all subchunks
                ph_vt, ph_g = [], []
                for s in range(nsub):
                    cl = slice(s * 128, s * 128 + 128)
                    ps_vt = pspool.tile([128, D], F32, tag="ps")
                    for j in range(4):
                        nc.tensor.matmul(
                            ps_vt[:], lhsT=xrf[:, j, cl], rhs=w2a_sb[:, j],
                            start=(j == 0), stop=False,
                        )
                    nc.tensor.matmul(
                        ps_vt[:], lhsT=ones_sb[:, 0:128], rhs=b2a_sb[:],
                        start=False, stop=True,
                    )
                    ps_g = pspool.tile([128, 128], F32, tag="ps")
                    nc.tensor.matmul(
                        ps_g[:], lhsT=a_k[:, cl], rhs=a_q[:, cl],
                        start=True, stop=True,
                    )
                    ph_vt.append(ps_vt)
                    ph_g.append(ps_g)
                # phase 2: ACT/DVE consumers for all subchunks
                ph_a2t, ph_p0 = [], []
                for s in range(nsub):
                    a2t = apool.tile([128, D], F32, tag="a2t")
                    nc.scalar.activation(a2t[:], ph_vt[s][:], AF.Relu)
                    e_t = apool.tile([128, 128], F32, tag="e")
                    nc.scalar.activation(e_t[:], ph_g[s][:], AF.Exp,
                                         bias=shift_sb[:])
                    p0 = apool.tile([128, 128], F32, tag="p0")
                    nc.vector.tensor_mul(p0[:], e_t[:], msk_sb[:])
                    ph_a2t.append(a2t)
                    ph_p0.append(p0)
                # phase 3: dependent matmuls + normalization per subchunk
                z_subs = []
                for s in range(nsub):
                    a2t, p0 = ph_a2t[s], ph_p0[s]
                    ps_z = pspool.tile([D, 128], F32, tag="ps")
                    nc.tensor.matmul(
                        ps_z[:], lhsT=a2t[:], rhs=p0[:], start=True, stop=True,
                    )
                    ps_d = pspool.tile([1, 128], F32, tag="ps")
                    nc.tensor.matmul(
                        ps_d[:], lhsT=ones_col[:], rhs=p0[:],
                        start=True, stop=True,
                    )
                    r_sb = apool.tile([1, 128], F32, tag="r")
                    nc.vector.reciprocal(r_sb[:], ps_d[:])
                    ps_r = pspool.tile([D, 128], F32, tag="ps")
                    nc.tensor.matmul(
                        ps_r[:], lhsT=ones_sb[:, 0:D], rhs=r_sb[:],
                        start=True, stop=True,
                    )
                    r64_sb = apool.tile([D, 128], F32, tag="r64")
                    nc.scalar.activation(r64_sb[:], ps_r[:], AF.Copy)
                    z_t = apool.tile([D, 4 * SUB], F32, tag="z")
                    nc.vector.tensor_mul(z_t[:], ps_z[:], r64_sb[:])
                    z_subs.append(z_t)

                # GEMM2 + bias; the w-broadcast happens in the matmul rhs
                # (step-0 AP re-reads each z column 4x) so the residual
                # add runs on plain stride-1 APs at full DVE rate.
                nsb = SUB
                for j in range(4):
                    for h in range(nsub):
                        nsl = slice(h * nsb, (h + 1) * nsb)
                        zv = (
                            z_subs[h][:]
                            .rearrange("p (n l) -> p n l", l=4)
                            .unsqueeze(3)
                            .broadcast_to((D, nsb, 4, 4))
                        )
                        ps_y = pypool.tile([128, 16 * nsb], F32, tag="psy")
                        nc.tensor.matmul(
                            ps_y[:], lhsT=w4t_sb[:, j], rhs=zv[:],
                            start=True, stop=True,
                        )
                        nc.vector.scalar_tensor_tensor(
                            xtv[:, nsl, j],
                            ps_y[:].rearrange("p (n h w) -> p n h w", h=4, w=4),
                            b4c_sb[:, j:j + 1],
                            xtv[:, nsl, j],
                            op0=mybir.AluOpType.add,
                            op1=mybir.AluOpType.add,
                        )

                # store on the scalar HWDGE queue to overlap with loads
                nc.scalar.dma_start(ov[b], x_t[:])

    nc.compile()
    _PROG_CACHE[key] = nc
    return nc


def prep_params(W123, b123, g123, be123, m123, v123, W4, b4, g4, be4, m4, v4):
    """Fold BN into the convs; permute channels for the c=4p+j packing."""
    f32 = np.float32
    s123 = (g123 / np.sqrt(v123 + EPS)).astype(f32)            # (3, D)
    Wf = (W123 * s123[:, :, None]).astype(f32)                 # (3, D, C)
    bf = ((b123 - m123) * s123 + be123).astype(f32)            # (3, D)
    s4 = (g4 / np.sqrt(v4 + EPS)).astype(f32)                  # (C,)
    W4f = (W4 * s4[:, None]).astype(f32)                       # (C, D)
    b4f = ((b4 - m4) * s4 + be4).astype(f32)                   # (C,)

    # perm[j*128 + p] = 4p + j : row j*128+p of a device weight tensor
    # holds original channel 4p+j (matching the x packing).
    p_idx, j_idx = np.meshgrid(np.arange(128), np.arange(4), indexing="ij")
    perm = (4 * p_idx + j_idx).T.reshape(-1)                   # (512,)

    wqk = np.concatenate([Wf[0].T, Wf[1].T], axis=1)[perm]     # (C, 128)
    bqk = np.concatenate([bf[0], bf[1]])[:, None]              # (128, 1)
    w2a = np.ascontiguousarray(Wf[2].T[perm])                  # (C, D)
    b2a = bf[2][None, :]                                       # (1, D)
    w4t = np.ascontiguousarray(W4f.T[:, perm])                 # (D, C)
    b4v = b4f[perm][None, :]                                   # (1, C)
    msk = np.kron(np.eye(SUB, dtype=f32), np.ones((4, 4), f32))  # (128, 128)
    return dict(
        wqk=np.ascontiguousarray(wqk), bqk=np.ascontiguousarray(bqk),
        w2a=w2a, b2a=np.ascontiguousarray(b2a),
        w4t=w4t, b4v=np.ascontiguousarray(b4v), msk=msk,
    )


def _run(inputs, trace=False, **spmd_kwargs):
    from concourse.bass_utils import run_bass_kernel_spmd

    x = np.ascontiguousarray(np.asarray(inputs["x"], dtype=np.float32))
    params = prep_params(**{k: np.asarray(v, np.float64)
                            for k, v in inputs.items() if k != "x"})
    nc = build_program()
    in_maps = [
        {"x": x[i * NSH:(i + 1) * NSH], **params} for i in range(NCORES)
    ]
    res = run_bass_kernel_spmd(
        nc, in_maps, list(range(NCORES)), trace=trace, **spmd_kwargs
    )
    outs = np.concatenate(
        [np.asarray(res.results[i]["out"]) for i in range(NCORES)], axis=0
    )
    return outs, res


def kernel(**inputs):
    outs, _ = _run(inputs)
    return outs



# revision 9
# speedup vs baseline: 1.4964x; 1.4964x over previous
"""Trainium2 Bass kernel for the attention-gate block (sample-major DMA).

Math (per sample n, after folding BN into the convs):
  X     = x[n, :, ::2, ::2].reshape(C, 4)                 # C=512, L=4
  act_k = relu(Wk' @ X + bk')            k=0,1,2          # D=64 each
  S     = act0^T act1  (4x4);  P = softmax_rows(S)
  Z     = P @ act2^T  (4x64)
  Y     = W4' @ Z^T + b4'                                  # (512, 4)
  out[n,c,h,w] = x[n,c,h,w] + Y[c,h]                       # broadcast over w

Device mapping (per core, 256 samples, blocks of 128):
  - SAMPLE-MAJOR DMA: partition = sample, so each partition line moves one
    8KB-contiguous chunk of a sample's row (4 chunk DMAs per block each
    way) -> line-rate HBM instead of 256B-packet descriptors.
  - the ::2,::2 gather + channel-major layout for the convs comes from 16
    PE transposes per block ([n,c]->[c,n] per (c-chunk k, position l)),
    evacuated to bf16 SBUF tiles; all conv/attention matmuls run in bf16.
  - attention: per 32-sample sub, one [64]x[128,128] gram matmul whose
    mask is tile(eye(32),(4,4)) under the (l-major, n-minor) column
    order; masked exp on ACT+DVE; denominators via a ones-column matmul,
    spread onto partitions as [sample, l] via 4 tiny outer-product
    matmuls per sub; ONE reciprocal for the whole block.
  - GEMM2 contracts over d with a bias row folded in (K=65), producing
    [c-chunk, sample] tiles that are PE-transposed back to sample-major;
    the softmax normalization rides the evacuation as a per-partition
    tensor_scalar multiply.
  - residual: one scalar_tensor_tensor per chunk adds Y (w-broadcast via
    a step-0 AP) into the fp32 x tile in place; stores go out on the
    scalar HWDGE queue while loads use the sync queue.
"""

import sys

for _p in ("/opt/trn_rl_repo",):
    if _p not in sys.path:
        sys.path.insert(0, _p)

import numpy as np
import ml_dtypes

import concourse.mybir as mybir
from concourse import bacc, tile

EPS = 1e-5
N_TOTAL, C, D, HH, WW = 2048, 512, 64, 4, 4
NCORES = 8
NSH = N_TOTAL // NCORES  # 256 samples per core
BLK = 128                # samples per block (= partition dim)
SUB = 32                 # samples per attention subchunk
NCH = 4                  # c-chunks of 128 channels
SHIFT = -34.0            # constant exp shift; cancels in the normalization
F32 = mybir.dt.float32
BF16 = mybir.dt.bfloat16

_PROG_CACHE = {}


def build_program(nsh=NSH, blk=BLK, reps=1):
    key = (nsh, blk, reps)
    if key in _PROG_CACHE:
        return _PROG_CACHE[key]
    assert blk == 128 and nsh % blk == 0

    nc = bacc.Bacc("TRN2", target_bir_lowering=False, debug=False)
    AF = mybir.ActivationFunctionType
    ADD = mybir.AluOpType.add

    x_in = nc.dram_tensor("x", (nsh, C, HH, WW), F32, kind="ExternalInput")
    wq = nc.dram_tensor("wq", (128, NCH, D), BF16, kind="ExternalInput")
    wk = nc.dram_tensor("wk", (128, NCH, D), BF16, kind="ExternalInput")
    w2 = nc.dram_tensor("w2", (128, NCH, D), BF16, kind="ExternalInput")
    w4a = nc.dram_tensor("w4a", (D + 1, NCH, 128), BF16, kind="ExternalInput")
    bq = nc.dram_tensor("bq", (D, 1), F32, kind="ExternalInput")
    bk = nc.dram_tensor("bk", (D, 1), F32, kind="ExternalInput")
    b2 = nc.dram_tensor("b2", (1, D), BF16, kind="ExternalInput")
    msk = nc.dram_tensor("msk", (128, 128), BF16, kind="ExternalInput")
    idn = nc.dram_tensor("idn", (128, 128), F32, kind="ExternalInput")
    ey4 = nc.dram_tensor("ey4", (1, 16), BF16, kind="ExternalInput")
    out = nc.dram_tensor("out", (nsh, C, HH, WW), F32, kind="ExternalOutput")

    nblk = nsh // blk
    CH = 2048  # elems per c-chunk of an x row: 128 c * 16 hw

    with tile.TileContext(nc) as tc:
        with (
            tc.tile_pool(name="const", bufs=1) as cpool,
            tc.tile_pool(name="xc", bufs=10) as xpool,
            tc.tile_pool(name="xsT", bufs=9) as tpool,
            tc.tile_pool(name="work", bufs=4) as wpool,
            tc.tile_pool(name="att", bufs=6) as apool,
            tc.tile_pool(name="ynm", bufs=6) as ypool,
            tc.tile_pool(name="ps", bufs=4, space="PSUM") as psA,
            tc.tile_pool(name="psr", bufs=1, space="PSUM") as psR,
            tc.tile_pool(name="pst", bufs=3, space="PSUM") as psT,
        ):
            wq_sb = cpool.tile([128, NCH, D], BF16)
            nc.sync.dma_start(wq_sb[:], wq[:])
            wk_sb = cpool.tile([128, NCH, D], BF16)
            nc.sync.dma_start(wk_sb[:], wk[:])
            w2_sb = cpool.tile([128, NCH, D], BF16)
            nc.sync.dma_start(w2_sb[:], w2[:])
            w4a_sb = cpool.tile([D + 1, NCH, 128], BF16)
            nc.sync.dma_start(w4a_sb[:], w4a[:])
            bq_sb = cpool.tile([D, 1], F32)
            nc.sync.dma_start(bq_sb[:], bq[:])
            bk_sb = cpool.tile([D, 1], F32)
            nc.sync.dma_start(bk_sb[:], bk[:])
            b2_sb = cpool.tile([1, D], BF16)
            nc.sync.dma_start(b2_sb[:], b2[:])
            msk_sb = cpool.tile([128, 128], BF16)
            nc.sync.dma_start(msk_sb[:], msk[:])
            idn_sb = cpool.tile([128, 128], F32)
            nc.sync.dma_start(idn_sb[:], idn[:])
            ey4_sb = cpool.tile([1, 16], BF16)
            nc.sync.dma_start(ey4_sb[:], ey4[:])
            ones_r = cpool.tile([1, 128], BF16)
            nc.vector.memset(ones_r[:], 1.0)
            ones_c = cpool.tile([128, 1], BF16)
            nc.vector.memset(ones_c[:], 1.0)
            shift_sb = cpool.tile([128, 1], F32)
            nc.vector.memset(shift_sb[:], SHIFT)

            # sample-major views: one 8KB-contiguous run per (sample, chunk)
            xv = x_in[:].rearrange("(b n) c h w -> b n (c h w)", n=blk)
            ov = out[:].rearrange("(b n) c h w -> b n (c h w)", n=blk)

            for b in [b for _ in range(reps) for b in range(nblk)]:
                # ---- load x chunks (sample-major, line-rate) ----
                x_c = []
                for k in range(NCH):
                    xt = xpool.tile([128, CH], F32, tag="xc")
                    nc.sync.dma_start(xt[:], xv[b, :, k * CH:(k + 1) * CH])
                    x_c.append(xt)

                # ---- transpose the ::2,::2 picks to channel-major bf16 ----
                # xsT[k] cols are (n-major, l-minor): [128c, 128n, 4l]
                xsT = []
                for k in range(NCH):
                    xst = tpool.tile([128, 128, 4], BF16, tag="xsT")
                    xcv = x_c[k][:].rearrange("p (c h w) -> p c h w", h=4, w=4)
                    for l in range(4):
                        hp, wp = (l // 2) * 2, (l % 2) * 2
                        ps_t = psT.tile([128, 128], F32, tag="pst")
                        nc.tensor.transpose(ps_t[:], xcv[:, :, hp, wp], idn_sb[:])
                        if l % 2 == 0:
                            nc.scalar.activation(xst[:, :, l], ps_t[:], AF.Copy)
                        else:
                            nc.vector.tensor_copy(xst[:, :, l], ps_t[:])
                    xsT.append(xst)

                # ---- GEMM1: q and k over 4 c-chunks ----
                ps_q = psA.tile([D, 512], F32, tag="ps")
                ps_k = psA.tile([D, 512], F32, tag="ps")
                for k in range(NCH):
                    xf = xsT[k][:].rearrange("p n l -> p (n l)")
                    nc.tensor.matmul(ps_q[:], lhsT=wq_sb[:, k], rhs=xf,
                                     start=(k == 0), stop=(k == 3))
                for k in range(NCH):
                    xf = xsT[k][:].rearrange("p n l -> p (n l)")
                    nc.tensor.matmul(ps_k[:], lhsT=wk_sb[:, k], rhs=xf,
                                     start=(k == 0), stop=(k == 3))
                a_q = wpool.tile([D, 512], BF16, tag="aq")
                nc.scalar.activation(a_q[:], ps_q[:], AF.Relu, bias=bq_sb[:])
                a_k = wpool.tile([D, 512], BF16, tag="ak")
                nc.scalar.activation(a_k[:], ps_k[:], AF.Relu, bias=bk_sb[:])

                # ---- attention per 32-sample sub ----
                # z_all rows 0..63 hold unnormalized P@V; row 64 holds the
                # softmax denominator per column, so GEMM2's K=65 contraction
                # yields W4@z + b4*d, and the 1/d evacuation scale recovers
                # W4@(z/d) + b4.
                z_all = apool.tile([D + 1, 4, 4, SUB], BF16, tag="z")  # (p, l, s, n)
                r_ps = psR.tile([128, 4], F32, tag="psr")
                for s in range(4):
                    ps_v = psA.tile([128, D], F32, tag="ps")
                    for k in range(NCH):
                        xl = (xsT[k][:, s * SUB:(s + 1) * SUB, :]
                              .rearrange("p n l -> p (n l)"))
                        nc.tensor.matmul(ps_v[:], lhsT=xl, rhs=w2_sb[:, k],
                                         start=(k == 0), stop=False)
                    nc.tensor.matmul(ps_v[:], lhsT=ones_r[:], rhs=b2_sb[:],
                                     start=False, stop=True)
                    a2t = apool.tile([128, D], BF16, tag="a2t")
                    nc.scalar.activation(a2t[:], ps_v[:], AF.Relu)

                    aks = a_k[:, s * 128:(s + 1) * 128]
                    aqs = a_q[:, s * 128:(s + 1) * 128]
                    ps_g = psA.tile([128, 128], F32, tag="ps")
                    nc.tensor.matmul(ps_g[:], lhsT=aks, rhs=aqs,
                                     start=True, stop=True)
                    e_t = apool.tile([128, 128], BF16, tag="e")
                    nc.scalar.activation(e_t[:], ps_g[:], AF.Exp,
                                         bias=shift_sb[:])
                    p0 = apool.tile([128, 128], BF16, tag="p0")
                    nc.vector.tensor_mul(p0[:], e_t[:], msk_sb[:])

                    ps_z = psA.tile([D, 128], F32, tag="ps")
                    nc.tensor.matmul(ps_z[:], lhsT=a2t[:], rhs=p0[:],
                                     start=True, stop=True)
                    nc.scalar.activation(
                        z_all[0:D, :, s, :].rearrange("p l n -> p n l"),
                        ps_z[:].rearrange("p (n l) -> p n l", l=4), AF.Copy)

                    ps_d = psA.tile([1, 128], F32, tag="ps")
                    nc.tensor.matmul(ps_d[:], lhsT=ones_c[:], rhs=p0[:],
                                     start=True, stop=True)
                    d_sb = apool.tile([1, 128], BF16, tag="d")
                    nc.scalar.activation(d_sb[:], ps_d[:], AF.Copy)
                    nc.scalar.activation(
                        z_all[D:D + 1, :, s, :].rearrange("p l n -> p n l"),
                        ps_d[:].rearrange("p (n l) -> p n l", l=4), AF.Copy)
                    # spread denominators onto partitions: [sample, l]
                    dv = d_sb[:].rearrange("p (n l) -> p n l", l=4)
                    for l in range(4):
                        nc.tensor.matmul(
                            r_ps[s * SUB:(s + 1) * SUB, :],
                            lhsT=dv[:, :, l],
                            rhs=ey4_sb[0:1, l * 4:(l + 1) * 4],
                            start=(l == 0), stop=(l == 3),
                            tile_position=(0, s * SUB),
                        )
                r_nm = wpool.tile([128, 4], F32, tag="r")
                nc.vector.reciprocal(r_nm[:], r_ps[:])

                # ---- GEMM2 (bias folded, K=65) + transpose back + norm ----
                for k in range(NCH):
                    y_nm = ypool.tile([128, 128, 4], BF16, tag="y")
                    for h in range(4):
                        ps_y = psT.tile([128, 128], F32, tag="pst")
                        nc.tensor.matmul(ps_y[:], lhsT=w4a_sb[:, k],
                                         rhs=z_all[:, h].rearrange(
                                             "p s n -> p (s n)"),
                                         start=True, stop=True)
                        y_sb = wpool.tile([128, 128], F32, tag="ysb")
                        nc.scalar.activation(y_sb[:], ps_y[:], AF.Copy)
                        ps_yt = psT.tile([128, 128], F32, tag="pst")
                        nc.tensor.transpose(ps_yt[:], y_sb[:], idn_sb[:])
                        if h % 2 == 0:
                            nc.vector.tensor_scalar_mul(
                                y_nm[:, :, h], ps_yt[:], r_nm[:, h:h + 1])
                        else:
                            nc.scalar.activation(
                                y_nm[:, :, h], ps_yt[:], AF.Copy,
                                scale=r_nm[:, h:h + 1])

                    # residual add with w-broadcast; store the chunk
                    xc4 = x_c[k][:].rearrange("p (c h w) -> p c h w", h=4, w=4)
                    ynb = y_nm[:].unsqueeze(3).broadcast_to((128, 128, 4, 4))
                    nc.vector.scalar_tensor_tensor(
                        xc4[:], ynb, 0.0, xc4[:], op0=ADD, op1=ADD)
                    nc.scalar.dma_start(ov[b, :, k * CH:(k + 1) * CH], x_c[k][:])

    nc.compile()
    _PROG_CACHE[key] = nc
    return nc


def prep_params(W123, b123, g123, be123, m123, v123, W4, b4, g4, be4, m4, v4):
    """Fold BN into the convs; cast to bf16 in the natural c order."""
    f32, bf = np.float32, ml_dtypes.bfloat16
    s123 = (g123 / np.sqrt(v123 + EPS)).astype(f32)            # (3, D)
    Wf = (W123 * s123[:, :, None]).astype(f32)                 # (3, D, C)
    bf123 = ((b123 - m123) * s123 + be123).astype(f32)         # (3, D)
    s4 = (g4 / np.sqrt(v4 + EPS)).astype(f32)                  # (C,)
    W4f = (W4 * s4[:, None]).astype(f32)                       # (C, D)
    b4f = ((b4 - m4) * s4 + be4).astype(f32)                   # (C,)

    def chunks(wt):  # (C, D) -> (128, NCH, D)
        return np.ascontiguousarray(
            wt.reshape(NCH, 128, D).transpose(1, 0, 2)).astype(bf)

    w4a = np.concatenate([W4f.T, b4f[None, :]], axis=0)        # (65, C)
    w4a = np.ascontiguousarray(
        w4a.reshape(D + 1, NCH, 128)).astype(bf)
    msk = np.kron(np.eye(SUB, dtype=f32), np.ones((4, 4), f32)).astype(bf)
    ey4 = np.eye(4, dtype=f32).reshape(1, 16).astype(bf)
    return dict(
        wq=chunks(Wf[0].T), wk=chunks(Wf[1].T), w2=chunks(Wf[2].T),
        w4a=w4a,
        bq=np.ascontiguousarray(bf123[0][:, None]).astype(f32),
        bk=np.ascontiguousarray(bf123[1][:, None]).astype(f32),
        b2=np.ascontiguousarray(bf123[2][None, :]).astype(bf),
        msk=msk, idn=np.eye(128, dtype=f32), ey4=ey4,
    )


def _run(inputs, trace=False, **spmd_kwargs):
    from concourse.bass_utils import run_bass_kernel_spmd

    x = np.ascontiguousarray(np.asarray(inputs["x"], dtype=np.float32))
    params = prep_params(**{k: np.asarray(v, np.float64)
                            for k, v in inputs.items() if k != "x"})
    nc = build_program()
    in_maps = [
        {"x": x[i * NSH:(i + 1) * NSH], **params} for i in range(NCORES)
    ]
    res = run_bass_kernel_spmd(
        nc, in_maps, list(range(NCORES)), trace=trace, **spmd_kwargs
    )
    outs = np.concatenate(
        [np.asarray(res.results[i]["out"]) for i in range(NCORES)], axis=0
    )
    return outs, res


def kernel(**inputs):
    outs, _ = _run(inputs)
    return outs


# revision 12
# speedup vs baseline: 1.6670x; 1.1140x over previous
"""Trainium2 Bass kernel for the attention-gate block (sample-major DMA).

Math (per sample n, after folding BN into the convs):
  X     = x[n, :, ::2, ::2].reshape(C, 4)                 # C=512, L=4
  act_k = relu(Wk' @ X + bk')            k=0,1,2          # D=64 each
  S     = act0^T act1  (4x4);  P = softmax_rows(S)
  Z     = P @ act2^T  (4x64)
  Y     = W4' @ Z^T + b4'                                  # (512, 4)
  out[n,c,h,w] = x[n,c,h,w] + Y[c,h]                       # broadcast over w

Device mapping (per core, 256 samples, blocks of 128):
  - SAMPLE-MAJOR DMA: partition = sample, so each partition line moves one
    8KB-contiguous chunk of a sample's row (4 chunk DMAs per block each
    way) -> line-rate HBM instead of 256B-packet descriptors.
  - the ::2,::2 gather + channel-major layout for the convs comes from 16
    PE transposes per block ([n,c]->[c,n] per (c-chunk k, position l)),
    evacuated to bf16 SBUF tiles; all conv/attention matmuls run in bf16.
  - attention: per 32-sample sub, one [64]x[128,128] gram matmul whose
    mask is tile(eye(32),(4,4)) under the (l-major, n-minor) column
    order; masked exp on ACT+DVE; denominators via a ones-column matmul,
    spread onto partitions as [sample, l] via 4 tiny outer-product
    matmuls per sub; ONE reciprocal for the whole block.
  - GEMM2 contracts over d with a bias row folded in (K=65), producing
    [c-chunk, sample] tiles that are PE-transposed back to sample-major;
    the softmax normalization rides the evacuation as a per-partition
    tensor_scalar multiply.
  - residual: one scalar_tensor_tensor per chunk adds Y (w-broadcast via
    a step-0 AP) into the fp32 x tile in place; stores go out on the
    scalar HWDGE queue while loads use the sync queue.
"""

import sys

for _p in ("/opt/trn_rl_repo",):
    if _p not in sys.path:
        sys.path.insert(0, _p)

import numpy as np
import ml_dtypes

import concourse.mybir as mybir
from concourse import bacc, tile

EPS = 1e-5
N_TOTAL, C, D, HH, WW = 2048, 512, 64, 4, 4
NCORES = 8
NSH = N_TOTAL // NCORES  # 256 samples per core
BLK = 128                # samples per block (= partition dim)
SUB = 32                 # samples per attention subchunk
NCH = 4                  # c-chunks of 128 channels
SHIFT = -34.0            # constant exp shift; cancels in the normalization
F32 = mybir.dt.float32
BF16 = mybir.dt.bfloat16

_PROG_CACHE = {}


def build_program(nsh=NSH, blk=BLK, reps=1):
    key = (nsh, blk, reps)
    if key in _PROG_CACHE:
        return _PROG_CACHE[key]
    assert blk == 128 and nsh % blk == 0

    nc = bacc.Bacc("TRN2", target_bir_lowering=False, debug=False)
    AF = mybir.ActivationFunctionType
    ADD = mybir.AluOpType.add

    x_in = nc.dram_tensor("x", (nsh, C, HH, WW), F32, kind="ExternalInput")
    wq = nc.dram_tensor("wq", (128, NCH, D), BF16, kind="ExternalInput")
    wk = nc.dram_tensor("wk", (128, NCH, D), BF16, kind="ExternalInput")
    w2 = nc.dram_tensor("w2", (128, NCH, D), BF16, kind="ExternalInput")
    w4a = nc.dram_tensor("w4a", (D + 1, NCH, 128), BF16, kind="ExternalInput")
    bq = nc.dram_tensor("bq", (D, 1), F32, kind="ExternalInput")
    bk = nc.dram_tensor("bk", (D, 1), F32, kind="ExternalInput")
    b2 = nc.dram_tensor("b2", (1, D), BF16, kind="ExternalInput")
    msk = nc.dram_tensor("msk", (128, 128), BF16, kind="ExternalInput")
    idn = nc.dram_tensor("idn", (128, 128), F32, kind="ExternalInput")
    ey4 = nc.dram_tensor("ey4", (1, 16), BF16, kind="ExternalInput")
    out = nc.dram_tensor("out", (nsh, C, HH, WW), F32, kind="ExternalOutput")

    nblk = nsh // blk
    CH = 2048  # elems per c-chunk of an x row: 128 c * 16 hw

    with tile.TileContext(nc) as tc:
        with (
            tc.tile_pool(name="const", bufs=1) as cpool,
            tc.tile_pool(name="xc", bufs=10) as xpool,
            tc.tile_pool(name="xsT", bufs=9) as tpool,
            tc.tile_pool(name="work", bufs=4) as wpool,
            tc.tile_pool(name="att", bufs=6) as apool,
            tc.tile_pool(name="ynm", bufs=6) as ypool,
            tc.tile_pool(name="ps", bufs=3, space="PSUM") as psA,
            tc.tile_pool(name="psr", bufs=1, space="PSUM") as psR,
            tc.tile_pool(name="pst", bufs=2, space="PSUM") as psT,
            tc.tile_pool(name="psy", bufs=2, space="PSUM") as psY,
        ):
            wq_sb = cpool.tile([128, NCH, D], BF16)
            nc.sync.dma_start(wq_sb[:], wq[:])
            wk_sb = cpool.tile([128, NCH, D], BF16)
            nc.sync.dma_start(wk_sb[:], wk[:])
            w2_sb = cpool.tile([128, NCH, D], BF16)
            nc.sync.dma_start(w2_sb[:], w2[:])
            w4a_sb = cpool.tile([D + 1, NCH, 128], BF16)
            nc.sync.dma_start(w4a_sb[:], w4a[:])
            bq_sb = cpool.tile([D, 1], F32)
            nc.sync.dma_start(bq_sb[:], bq[:])
            bk_sb = cpool.tile([D, 1], F32)
            nc.sync.dma_start(bk_sb[:], bk[:])
            b2_sb = cpool.tile([1, D], BF16)
            nc.sync.dma_start(b2_sb[:], b2[:])
            msk_sb = cpool.tile([128, 128], BF16)
            nc.sync.dma_start(msk_sb[:], msk[:])
            idn_sb = cpool.tile([128, 128], F32)
            nc.sync.dma_start(idn_sb[:], idn[:])
            ey4_sb = cpool.tile([1, 16], BF16)
            nc.sync.dma_start(ey4_sb[:], ey4[:])
            ones_r = cpool.tile([1, 128], BF16)
            nc.vector.memset(ones_r[:], 1.0)
            ones_c = cpool.tile([128, 1], BF16)
            nc.vector.memset(ones_c[:], 1.0)
            shift_sb = cpool.tile([128, 1], F32)
            nc.vector.memset(shift_sb[:], SHIFT)

            # sample-major views: one 8KB-contiguous run per (sample, chunk)
            xv = x_in[:].rearrange("(b n) c h w -> b n (c h w)", n=blk)
            ov = out[:].rearrange("(b n) c h w -> b n (c h w)", n=blk)

            for b in [b for _ in range(reps) for b in range(nblk)]:
                # ---- load x chunks (sample-major, line-rate) ----
                x_c = []
                for k in range(NCH):
                    xt = xpool.tile([128, CH], F32, tag="xc")
                    nc.sync.dma_start(xt[:], xv[b, :, k * CH:(k + 1) * CH])
                    x_c.append(xt)

                # ---- transpose the ::2,::2 picks to channel-major bf16 ----
                # xsT[k] cols are (n-major, l-minor): [128c, 128n, 4l]
                xsT = []
                for k in range(NCH):
                    xst = tpool.tile([128, 128, 4], BF16, tag="xsT")
                    xcv = x_c[k][:].rearrange("p (c h w) -> p c h w", h=4, w=4)
                    ps_t = psT.tile([128, 4, 128], F32, tag="pst")
                    for l in range(4):
                        hp, wp = (l // 2) * 2, (l % 2) * 2
                        nc.tensor.transpose(ps_t[:, l], xcv[:, :, hp, wp],
                                            idn_sb[:])
                    xtv = xst[:].rearrange("p n l -> p l n")
                    if k % 2 == 0:
                        nc.scalar.activation(xtv, ps_t[:], AF.Copy)
                    else:
                        nc.vector.tensor_copy(xtv, ps_t[:])
                    xsT.append(xst)

                # ---- GEMM1: q and k over 4 c-chunks ----
                ps_q = psA.tile([D, 512], F32, tag="ps")
                ps_k = psA.tile([D, 512], F32, tag="ps")
                for k in range(NCH):
                    xf = xsT[k][:].rearrange("p n l -> p (n l)")
                    nc.tensor.matmul(ps_q[:], lhsT=wq_sb[:, k], rhs=xf,
                                     start=(k == 0), stop=(k == 3))
                for k in range(NCH):
                    xf = xsT[k][:].rearrange("p n l -> p (n l)")
                    nc.tensor.matmul(ps_k[:], lhsT=wk_sb[:, k], rhs=xf,
                                     start=(k == 0), stop=(k == 3))
                a_q = wpool.tile([D, 512], BF16, tag="aq")
                nc.scalar.activation(a_q[:], ps_q[:], AF.Relu, bias=bq_sb[:])
                a_k = wpool.tile([D, 512], BF16, tag="ak")
                nc.scalar.activation(a_k[:], ps_k[:], AF.Relu, bias=bk_sb[:])

                # ---- attention per 32-sample sub ----
                # z_all rows 0..63 hold unnormalized P@V; row 64 holds the
                # softmax denominator per column, so GEMM2's K=65 contraction
                # yields W4@z + b4*d, and the 1/d evacuation scale recovers
                # W4@(z/d) + b4.
                z_all = apool.tile([D + 1, 4, 4, SUB], BF16, tag="z")  # (p, l, s, n)
                r_ps = psR.tile([128, 4], F32, tag="psr")
                for s in range(4):
                    ps_v = psA.tile([128, D], F32, tag="ps")
                    for k in range(NCH):
                        xl = (xsT[k][:, s * SUB:(s + 1) * SUB, :]
                              .rearrange("p n l -> p (n l)"))
                        nc.tensor.matmul(ps_v[:], lhsT=xl, rhs=w2_sb[:, k],
                                         start=(k == 0), stop=False)
                    nc.tensor.matmul(ps_v[:], lhsT=ones_r[:], rhs=b2_sb[:],
                                     start=False, stop=True)
                    a2t = apool.tile([128, D], BF16, tag="a2t")
                    nc.scalar.activation(a2t[:], ps_v[:], AF.Relu)

                    aks = a_k[:, s * 128:(s + 1) * 128]
                    aqs = a_q[:, s * 128:(s + 1) * 128]
                    ps_g = psA.tile([128, 128], F32, tag="ps")
                    nc.tensor.matmul(ps_g[:], lhsT=aks, rhs=aqs,
                                     start=True, stop=True)
                    e_t = apool.tile([128, 128], BF16, tag="e")
                    nc.scalar.activation(e_t[:], ps_g[:], AF.Exp,
                                         bias=shift_sb[:])
                    p0 = apool.tile([128, 128], BF16, tag="p0")
                    nc.vector.tensor_mul(p0[:], e_t[:], msk_sb[:])

                    ps_z = psA.tile([D, 128], F32, tag="ps")
                    nc.tensor.matmul(ps_z[:], lhsT=a2t[:], rhs=p0[:],
                                     start=True, stop=True)
                    nc.scalar.activation(
                        z_all[0:D, :, s, :].rearrange("p l n -> p n l"),
                        ps_z[:].rearrange("p (n l) -> p n l", l=4), AF.Copy)

                    ps_d = psA.tile([1, 128], F32, tag="ps")
                    nc.tensor.matmul(ps_d[:], lhsT=ones_c[:], rhs=p0[:],
                                     start=True, stop=True)
                    d_sb = apool.tile([1, 128], BF16, tag="d")
                    nc.scalar.activation(d_sb[:], ps_d[:], AF.Copy)
                    nc.scalar.activation(
                        z_all[D:D + 1, :, s, :].rearrange("p l n -> p n l"),
                        ps_d[:].rearrange("p (n l) -> p n l", l=4), AF.Copy)
                    # spread denominators onto partitions: [sample, l]
                    dv = d_sb[:].rearrange("p (n l) -> p n l", l=4)
                    for l in range(4):
                        nc.tensor.matmul(
                            r_ps[s * SUB:(s + 1) * SUB, :],
                            lhsT=dv[:, :, l],
                            rhs=ey4_sb[0:1, l * 4:(l + 1) * 4],
                            start=(l == 0), stop=(l == 3),
                            tile_position=(0, s * SUB),
                        )
                r_nm = wpool.tile([128, 4], F32, tag="r")
                nc.vector.reciprocal(r_nm[:], r_ps[:])

                # ---- GEMM2: z slice stationary -> sample-major Y directly,
                # softmax normalization folded into the evacuation scale ----
                w4f = w4a_sb[:].rearrange("p k c -> p (k c)")
                y_all = ypool.tile([128, 512, 4], BF16, tag="y")
                for h in range(4):
                    ps_y = psY.tile([128, 512], F32, tag="psy")
                    nc.tensor.matmul(
                        ps_y[:], lhsT=z_all[:, h].rearrange("p s n -> p (s n)"),
                        rhs=w4f, start=True, stop=True)
                    if h % 2 == 0:
                        nc.vector.tensor_scalar_mul(
                            y_all[:, :, h], ps_y[:], r_nm[:, h:h + 1])
                    else:
                        nc.scalar.activation(
                            y_all[:, :, h], ps_y[:], AF.Copy,
                            scale=r_nm[:, h:h + 1])

                # ---- residual add with w-broadcast; store each chunk ----
                for k in range(NCH):
                    xc4 = x_c[k][:].rearrange("p (c h w) -> p c h w", h=4, w=4)
                    ynb = (y_all[:, k * 128:(k + 1) * 128, :]
                           .unsqueeze(3).broadcast_to((128, 128, 4, 4)))
                    eng = nc.vector if k < 2 else nc.gpsimd
                    eng.tensor_add(xc4[:], ynb, xc4[:])
                    nc.scalar.dma_start(ov[b, :, k * CH:(k + 1) * CH], x_c[k][:])

    nc.compile()
    _PROG_CACHE[key] = nc
    return nc


def prep_params(W123, b123, g123, be123, m123, v123, W4, b4, g4, be4, m4, v4):
    """Fold BN into the convs; cast to bf16 in the natural c order."""
    f32, bf = np.float32, ml_dtypes.bfloat16
    s123 = (g123 / np.sqrt(v123 + EPS)).astype(f32)            # (3, D)
    Wf = (W123 * s123[:, :, None]).astype(f32)                 # (3, D, C)
    bf123 = ((b123 - m123) * s123 + be123).astype(f32)         # (3, D)
    s4 = (g4 / np.sqrt(v4 + EPS)).astype(f32)                  # (C,)
    W4f = (W4 * s4[:, None]).astype(f32)                       # (C, D)
    b4f = ((b4 - m4) * s4 + be4).astype(f32)                   # (C,)

    def chunks(wt):  # (C, D) -> (128, NCH, D)
        return np.ascontiguousarray(
            wt.reshape(NCH, 128, D).transpose(1, 0, 2)).astype(bf)

    w4a = np.concatenate([W4f.T, b4f[None, :]], axis=0)        # (65, C)
    w4a = np.ascontiguousarray(
        w4a.reshape(D + 1, NCH, 128)).astype(bf)
    msk = np.kron(np.eye(SUB, dtype=f32), np.ones((4, 4), f32)).astype(bf)
    ey4 = np.eye(4, dtype=f32).reshape(1, 16).astype(bf)
    return dict(
        wq=chunks(Wf[0].T), wk=chunks(Wf[1].T), w2=chunks(Wf[2].T),
        w4a=w4a,
        bq=np.ascontiguousarray(bf123[0][:, None]).astype(f32),
        bk=np.ascontiguousarray(bf123[1][:, None]).astype(f32),
        b2=np.ascontiguousarray(bf123[2][None, :]).astype(bf),
        msk=msk, idn=np.eye(128, dtype=f32), ey4=ey4,
    )


def _run(inputs, trace=False, **spmd_kwargs):
    from concourse.bass_utils import run_bass_kernel_spmd

    x = np.ascontiguousarray(np.asarray(inputs["x"], dtype=np.float32))
    params = prep_params(**{k: np.asarray(v, np.float64)
                            for k, v in inputs.items() if k != "x"})
    nc = build_program()
    in_maps = [
        {"x": x[i * NSH:(i + 1) * NSH], **params} for i in range(NCORES)
    ]
    res = run_bass_kernel_spmd(
        nc, in_maps, list(range(NCORES)), trace=trace, **spmd_kwargs
    )
    outs = np.concatenate(
        [np.asarray(res.results[i]["out"]) for i in range(NCORES)], axis=0
    )
    return outs, res


def kernel(**inputs):
    outs, _ = _run(inputs)
    return outs
